# revision 1
# baseline (speedup 1.0000x reference)
"""MixtralMoE (dense top-2, 8 experts) on 8 trn2 NeuronCores — expert parallel.

Layout strategy: everything on-device runs "transposed" (hidden/FFN on
partitions, tokens on the free axis), so every matmul's stationary operand is
a natural slice of the weights and the streaming operand is xT / hT.

Per core c (= expert c):
  phase 0 (fp32): router logits per 128-token tile, top-2 + renormalize via
    max/exp/mask vector ops (the softmax denominator cancels), extract column
    c, PE-transpose + K=1 ones-matmul into a [128, 4096] broadcast of the
    per-token weight.
  main loop over 8 token blocks of 512:
    hT[f, :]  = silu(w1[:, f].T @ xT) * (w3[:, f].T @ xT)   (bf16, f32 accum)
    outT[m,:] = sum_f w2[f, m].T @ hT[f, :]                  (f32 accum)
    evacuate PSUM through a DVE multiply with the routing-weight broadcast,
    DMA to DRAM, ReduceScatter(add) over the 8 cores on the H axis.
Host: concat the 8 [256, 4096] shards -> outT [2048, 4096] -> transpose.
"""

import numpy as np
import ml_dtypes

NUM_EXPERTS = 8
TOP_K = 2
HIDDEN = 2048
FFN = 7168
TOKENS = 4096
N_CORES = 8

T_BLK = 512
N_BLOCKS = TOKENS // T_BLK          # 8
KH = HIDDEN // 128                  # 16 k-chunks for hidden
KF = FFN // 128                     # 56 chunks for ffn
H_SHARD = HIDDEN // N_CORES         # 256 rows of outT per core after RS

USE_RS = True


def _apply_bir_patch():
    """This container's walrus build rejects >1 semaphore wait per
    instruction ("Too many sync wait commands").  Split multi-wait
    instructions at the BIR-JSON level into single-wait NoOps on the same
    engine (sequential single waits on one engine are equivalent)."""
    import orjson
    from concourse import bass_utils

    if getattr(bass_utils, "_multiwait_patch", False):
        return
    bass_utils._multiwait_patch = True

    orig = bass_utils.compile_bir_kernel

    def split_multiwait(bir):
        ctr = 0
        for fn in bir.get("functions", []):
            for blk in fn.get("blocks", []):
                out = []
                changed = False
                for ins in blk.get("instructions", []):
                    si = ins.get("sync_info")
                    ow = (si or {}).get("on_wait") or []
                    if len(ow) > 1:
                        for w in ow[:-1]:
                            out.append({
                                "debug": ins.get("debug", 0),
                                "engine": ins["engine"],
                                "ins": [],
                                "name": f"swsplit-{ctr}",
                                "opcode": "NoOp",
                                "outs": [],
                                "sync_info": {"on_update": [], "on_wait": [w]},
                            })
                            ctr += 1
                        si["on_wait"] = [ow[-1]]
                        changed = True
                    out.append(ins)
                if changed:
                    blk["instructions"] = out
        return bir

    def patched(bir_json, tmpdir, neff_name="file.neff"):
        bir = split_multiwait(orjson.loads(bir_json))
        return orig(orjson.dumps(bir), tmpdir, neff_name=neff_name)

    bass_utils.compile_bir_kernel = patched
    try:
        from concourse import bass2jax
        if getattr(bass2jax, "compile_bir_kernel", None) is orig:
            bass2jax.compile_bir_kernel = patched
    except ImportError:
        pass


def build():
    import concourse.bass as bass
    import concourse.mybir as mybir
    import concourse.tile as tile
    from concourse.masks import make_identity

    f32 = mybir.dt.float32
    bf16 = mybir.dt.bfloat16
    AF = mybir.ActivationFunctionType
    ALU = mybir.AluOpType

    nc = bass.Bass(num_devices=N_CORES)

    xtf = nc.declare_dram_parameter("xtf", [HIDDEN, TOKENS], f32, isOutput=False)
    xtb = nc.declare_dram_parameter("xtb", [HIDDEN, TOKENS], bf16, isOutput=False)
    gw = nc.declare_dram_parameter("gw", [HIDDEN, NUM_EXPERTS], f32, isOutput=False)
    sel = nc.declare_dram_parameter("sel", [128, NUM_EXPERTS], f32, isOutput=False)
    w1 = nc.declare_dram_parameter("w1", [HIDDEN, FFN], bf16, isOutput=False)
    w3 = nc.declare_dram_parameter("w3", [HIDDEN, FFN], bf16, isOutput=False)
    w2 = nc.declare_dram_parameter("w2", [FFN, HIDDEN], bf16, isOutput=False)
    if USE_RS:
        y = nc.declare_dram_parameter("y", [H_SHARD, TOKENS], f32, isOutput=True)
    else:
        y = nc.declare_dram_parameter("y", [HIDDEN, TOKENS], f32, isOutput=True)

    # DRAM-side rearranged views (partition-first) for strided loads.
    xtf_v = xtf.rearrange("(k p) t -> p k t", p=128)     # [128, KH, TOKENS]
    xtb_v = xtb.rearrange("(k p) t -> p k t", p=128)
    gw_v = gw.rearrange("(k p) e -> p k e", p=128)       # [128, KH, 8]
    w1_v = w1.rearrange("(k p) m -> p k m", p=128)       # [128, KH, FFN]
    w3_v = w3.rearrange("(k p) m -> p k m", p=128)
    w2_v = w2.rearrange("(k p) m -> p k m", p=128)       # [128, KF, HIDDEN]

    with tile.TileContext(nc) as tc:
        with tc.tile_pool(name="persist", bufs=1) as persist, \
             tc.tile_pool(name="dram", bufs=2, space="DRAM") as dram:

            # ---- persistent tiles ----
            wb_sb = persist.tile([128, TOKENS], f32, tag="wb")      # weight bcast
            hT = persist.tile([128, KF * T_BLK], bf16, tag="hT")    # [f-chunks x tokens]
            ident = persist.tile([128, 128], f32, tag="ident")
            make_identity(nc, ident[:])
            ones1 = persist.tile([1, 128], f32, tag="ones1")
            nc.vector.memset(ones1[:], 1.0)
            gwt = persist.tile([128, KH * NUM_EXPERTS], f32, tag="gwt")
            nc.sync.dma_start(gwt.rearrange("p (k e) -> p k e", k=KH), gw_v[:])
            sel_sb = persist.tile([128, NUM_EXPERTS], f32, tag="sel")
            nc.sync.dma_start(sel_sb[:], sel[:])
            wrow = persist.tile([1, TOKENS], f32, tag="wrow")

            # ================= phase 0: router =================
            with tc.tile_pool(name="rt", bufs=2) as rt, \
                 tc.tile_pool(name="rtp", bufs=2, space="PSUM") as rtp:
                for t in range(TOKENS // 128):
                    xf = rt.tile([128, KH * 128], f32, tag="xf")
                    nc.sync.dma_start(
                        xf.rearrange("p (k c) -> p k c", k=KH),
                        xtf_v[:, :, t * 128:(t + 1) * 128])
                    lp = rtp.tile([128, NUM_EXPERTS], f32, tag="lp")
                    for k in range(KH):
                        nc.tensor.matmul(
                            lp[:], xf[:, k * 128:(k + 1) * 128],
                            gwt[:, k * NUM_EXPERTS:(k + 1) * NUM_EXPERTS],
                            start=(k == 0), stop=(k == KH - 1))
                    ls = rt.tile([128, NUM_EXPERTS], f32, tag="ls")
                    nc.vector.tensor_copy(ls[:], lp[:])
                    m1 = rt.tile([128, 1], f32, tag="m1")
                    nc.vector.reduce_max(m1[:], ls[:], axis=mybir.AxisListType.X)
                    ge1 = rt.tile([128, NUM_EXPERTS], f32, tag="ge1")
                    nc.vector.tensor_scalar(ge1[:], ls[:], m1[:], None, ALU.is_ge)
                    ls2 = rt.tile([128, NUM_EXPERTS], f32, tag="ls2")
                    # ls2 = ge1 * -1e30 + ls   (mask out the top-1 entry)
                    nc.vector.scalar_tensor_tensor(
                        ls2[:], ge1[:], -1.0e30, ls[:], op0=ALU.mult, op1=ALU.add)
                    m2 = rt.tile([128, 1], f32, tag="m2")
                    nc.vector.reduce_max(m2[:], ls2[:], axis=mybir.AxisListType.X)
                    keep = rt.tile([128, NUM_EXPERTS], f32, tag="keep")
                    nc.vector.tensor_scalar(keep[:], ls[:], m2[:], None, ALU.is_ge)
                    m1n = rt.tile([128, 1], f32, tag="m1n")
                    nc.vector.tensor_scalar_mul(m1n[:], m1[:], -1.0)
                    e1 = rt.tile([128, NUM_EXPERTS], f32, tag="e1")
                    nc.scalar.activation(e1[:], ls[:], AF.Exp, bias=m1n[:])
                    ek = rt.tile([128, NUM_EXPERTS], f32, tag="ek")
                    nc.vector.tensor_mul(ek[:], e1[:], keep[:])
                    den = rt.tile([128, 1], f32, tag="den")
                    nc.vector.reduce_sum(den[:], ek[:], axis=mybir.AxisListType.X)
                    rec = rt.tile([128, 1], f32, tag="rec")
                    nc.vector.reciprocal(rec[:], den[:])
                    ws = rt.tile([128, NUM_EXPERTS], f32, tag="ws")
                    nc.vector.tensor_mul(ws[:], ek[:], sel_sb[:])
                    wcu = rt.tile([128, 1], f32, tag="wcu")
                    nc.vector.reduce_sum(wcu[:], ws[:], axis=mybir.AxisListType.X)
                    wcol = rt.tile([128, 1], f32, tag="wcol")
                    nc.vector.tensor_mul(wcol[:], wcu[:], rec[:])
                    tp = rtp.tile([128, 128], f32, tag="tp")
                    nc.tensor.transpose(tp[:1, :128], wcol[:], ident[:])
                    nc.vector.tensor_copy(wrow[:1, t * 128:(t + 1) * 128], tp[:1, :128])
                # broadcast wrow [1, TOKENS] -> wb_sb [128, TOKENS]
                for s in range(TOKENS // 512):
                    bc = rtp.tile([128, 512], f32, tag="bc")
                    nc.tensor.matmul(
                        bc[:], ones1[:], wrow[:1, s * 512:(s + 1) * 512],
                        start=True, stop=True)
                    nc.vector.tensor_copy(wb_sb[:, s * 512:(s + 1) * 512], bc[:])

            # ================= main loop =================
            with tc.tile_pool(name="xb", bufs=2) as xbp, \
                 tc.tile_pool(name="w13", bufs=4) as w13p, \
                 tc.tile_pool(name="w2p", bufs=2) as w2p, \
                 tc.tile_pool(name="tmp", bufs=2) as tmp, \
                 tc.tile_pool(name="oev", bufs=3) as oev, \
                 tc.tile_pool(name="mps", bufs=2, space="PSUM") as mps:
                for b in range(N_BLOCKS):
                    tsl = slice(b * T_BLK, (b + 1) * T_BLK)
                    xb = xbp.tile([128, KH * T_BLK], bf16, tag="xb")
                    nc.sync.dma_start(
                        xb.rearrange("p (k t) -> p k t", k=KH),
                        xtb_v[:, :, tsl])
                    # ---- h phase ----
                    for f in range(KF):
                        fsl = slice(f * 128, (f + 1) * 128)
                        w1t = w13p.tile([128, KH * 128], bf16, tag="w13")
                        nc.sync.dma_start(
                            w1t.rearrange("p (k c) -> p k c", k=KH),
                            w1_v[:, :, fsl])
                        w3t = w13p.tile([128, KH * 128], bf16, tag="w13")
                        nc.sync.dma_start(
                            w3t.rearrange("p (k c) -> p k c", k=KH),
                            w3_v[:, :, fsl])
                        h1p = mps.tile([128, T_BLK], f32, tag="h1")
                        h3p = mps.tile([128, T_BLK], f32, tag="h3")
                        for k in range(KH):
                            ksl = slice(k * 128, (k + 1) * 128)
                            xsl = slice(k * T_BLK, (k + 1) * T_BLK)
                            nc.tensor.matmul(h1p[:], w1t[:, ksl], xb[:, xsl],
                                             start=(k == 0), stop=(k == KH - 1))
                        for k in range(KH):
                            ksl = slice(k * 128, (k + 1) * 128)
                            xsl = slice(k * T_BLK, (k + 1) * T_BLK)
                            nc.tensor.matmul(h3p[:], w3t[:, ksl], xb[:, xsl],
                                             start=(k == 0), stop=(k == KH - 1))
                        h1s = tmp.tile([128, T_BLK], f32, tag="h1s")
                        nc.scalar.activation(h1s[:], h1p[:], AF.Silu)
                        nc.vector.tensor_mul(
                            hT[:, f * T_BLK:(f + 1) * T_BLK], h1s[:], h3p[:])
                    # ---- w2 phase ----
                    outblk = dram.tile([HIDDEN, T_BLK], f32, tag="outblk")
                    for m in range(KH):
                        msl = slice(m * 128, (m + 1) * 128)
                        w2t = w2p.tile([128, KF * 128], bf16, tag="w2")
                        nc.sync.dma_start(
                            w2t.rearrange("p (k c) -> p k c", k=KF),
                            w2_v[:, :, msl])
                        op = mps.tile([128, T_BLK], f32, tag="o")
                        for kf in range(KF):
                            nc.tensor.matmul(
                                op[:], w2t[:, kf * 128:(kf + 1) * 128],
                                hT[:, kf * T_BLK:(kf + 1) * T_BLK],
                                start=(kf == 0), stop=(kf == KF - 1))
                        os = oev.tile([128, T_BLK], f32, tag="os")
                        nc.vector.tensor_mul(os[:], op[:], wb_sb[:, tsl])
                        nc.sync.dma_start(outblk[msl, :], os[:])
                    if USE_RS:
                        rsout = dram.tile([H_SHARD, T_BLK], f32, tag="rsout")
                        nc.gpsimd.collective_compute(
                            "ReduceScatter", mybir.AluOpType.add,
                            replica_groups=[list(range(N_CORES))],
                            ins=[outblk.opt()], outs=[rsout.opt()])
                        nc.sync.dma_start(y[:, tsl], rsout[:])
                    else:
                        nc.sync.dma_start(y[:, tsl], outblk[:])
    return nc


_CACHED = {}


def _get_nc():
    if "nc" not in _CACHED:
        _apply_bir_patch()
        _CACHED["nc"] = build()
    return _CACHED["nc"]


def _prep_in_maps(inputs):
    bf16 = ml_dtypes.bfloat16
    x = np.ascontiguousarray(np.asarray(inputs["hidden_states"], dtype=np.float32))
    gate = np.ascontiguousarray(np.asarray(inputs["gate_w"], dtype=np.float32))
    w1s = np.asarray(inputs["w1s"])
    w3s = np.asarray(inputs["w3s"])
    w2s = np.asarray(inputs["w2s"])

    xtf = np.ascontiguousarray(x.T)                       # [H, T] f32
    xtb = np.ascontiguousarray(xtf.astype(bf16))          # [H, T] bf16

    in_maps = []
    for c in range(N_CORES):
        sel = np.zeros((128, NUM_EXPERTS), dtype=np.float32)
        sel[:, c] = 1.0
        in_maps.append({
            "xtf": xtf,
            "xtb": xtb,
            "gw": gate,
            "sel": sel,
            "w1": np.ascontiguousarray(w1s[c].astype(bf16)),
            "w3": np.ascontiguousarray(w3s[c].astype(bf16)),
            "w2": np.ascontiguousarray(w2s[c].astype(bf16)),
        })
    return in_maps


def run(inputs, trace=False, trace_cores=None):
    """Build + run; returns (full_output [T, H] f32, BassKernelResults)."""
    from concourse.bass_utils import run_bass_kernel_spmd

    nc = _get_nc()
    in_maps = _prep_in_maps(inputs)
    res = run_bass_kernel_spmd(
        nc, in_maps, list(range(N_CORES)), trace=trace,
        **({"trace_cores": trace_cores} if trace_cores else {}))
    if USE_RS:
        outT = np.concatenate([res.results[c]["y"] for c in range(N_CORES)], axis=0)
    else:
        outT = np.sum([res.results[c]["y"] for c in range(N_CORES)], axis=0)
    out = np.ascontiguousarray(outT.T.astype(np.float32))
    return out, res


def kernel(**inputs) -> np.ndarray:
    out, _ = run(inputs, trace=False)
    return out


# revision 6
# speedup vs baseline: 1.0335x; 1.0335x over previous
"""MixtralMoE (dense top-2, 8 experts) on 8 trn2 NeuronCores — expert parallel.

Layout strategy: everything on-device runs "transposed" (hidden/FFN on
partitions, tokens on the free axis), so every matmul's stationary operand is
a natural slice of the weights and the streaming operand is xT / hT.

Per core c (= expert c):
  phase 0 (fp32): router logits per 128-token tile, top-2 + renormalize via
    max/exp/mask vector ops (the softmax denominator cancels), extract column
    c, PE-transpose + K=1 ones-matmul into a [128, 4096] broadcast of the
    per-token weight.
  main loop over 8 token blocks of 512:
    hT[f, :]  = silu(w1[:, f].T @ xT) * (w3[:, f].T @ xT)   (bf16, f32 accum)
    outT[m,:] = sum_f w2[f, m].T @ hT[f, :]                  (f32 accum)
    evacuate PSUM through a DVE multiply with the routing-weight broadcast,
    DMA to DRAM, ReduceScatter(add) over the 8 cores on the H axis.
Host: concat the 8 [256, 4096] shards -> outT [2048, 4096] -> transpose.
"""

import os

import numpy as np
import ml_dtypes

NUM_EXPERTS = 8
TOP_K = 2
HIDDEN = 2048
FFN = 7168
TOKENS = 4096
N_CORES = 8

T_BLK = 512
N_BLOCKS = TOKENS // T_BLK          # 8
KH = HIDDEN // 128                  # 16 k-chunks for hidden
KF = FFN // 128                     # 56 chunks for ffn
H_SHARD = HIDDEN // N_CORES         # 256 rows of outT per core after RS

USE_RS = True


def _apply_bir_patch():
    """This container's walrus build rejects >1 semaphore wait per
    instruction ("Too many sync wait commands").  Split multi-wait
    instructions at the BIR-JSON level into single-wait NoOps on the same
    engine (sequential single waits on one engine are equivalent)."""
    import orjson
    from concourse import bass_utils

    if getattr(bass_utils, "_multiwait_patch", False):
        return
    bass_utils._multiwait_patch = True

    orig = bass_utils.compile_bir_kernel

    def split_multiwait(bir):
        ctr = 0
        for fn in bir.get("functions", []):
            for blk in fn.get("blocks", []):
                out = []
                changed = False
                for ins in blk.get("instructions", []):
                    si = ins.get("sync_info")
                    ow = (si or {}).get("on_wait") or []
                    if len(ow) > 1:
                        for w in ow[:-1]:
                            out.append({
                                "debug": ins.get("debug", 0),
                                "engine": ins["engine"],
                                "ins": [],
                                "name": f"swsplit-{ctr}",
                                "opcode": "NoOp",
                                "outs": [],
                                "sync_info": {"on_update": [], "on_wait": [w]},
                            })
                            ctr += 1
                        si["on_wait"] = [ow[-1]]
                        changed = True
                    out.append(ins)
                if changed:
                    blk["instructions"] = out
        return bir

    def patched(bir_json, tmpdir, neff_name="file.neff"):
        bir = split_multiwait(orjson.loads(bir_json))
        return orig(orjson.dumps(bir), tmpdir, neff_name=neff_name)

    bass_utils.compile_bir_kernel = patched
    try:
        from concourse import bass2jax
        if getattr(bass2jax, "compile_bir_kernel", None) is orig:
            bass2jax.compile_bir_kernel = patched
    except ImportError:
        pass

    if os.environ.get("KERNEL_LDW_OPT"):
        orig_run = bass_utils.run_command

        def patched_run(cmd, **kw):
            cmd = [a.replace("--enable-ldw-opt=false", "--enable-ldw-opt=true")
                   if isinstance(a, str) else a for a in cmd]
            return orig_run(cmd, **kw)

        bass_utils.run_command = patched_run


def build():
    import concourse.bass as bass
    import concourse.mybir as mybir
    import concourse.tile as tile
    from concourse.masks import make_identity

    f32 = mybir.dt.float32
    bf16 = mybir.dt.bfloat16
    AF = mybir.ActivationFunctionType
    ALU = mybir.AluOpType

    nc = bass.Bass(num_devices=N_CORES)

    xtf = nc.declare_dram_parameter("xtf", [HIDDEN, TOKENS], f32, isOutput=False)
    xtb = nc.declare_dram_parameter("xtb", [HIDDEN, TOKENS], bf16, isOutput=False)
    gw = nc.declare_dram_parameter("gw", [HIDDEN, NUM_EXPERTS], f32, isOutput=False)
    sel = nc.declare_dram_parameter("sel", [128, NUM_EXPERTS], f32, isOutput=False)
    w1 = nc.declare_dram_parameter("w1", [HIDDEN, FFN], bf16, isOutput=False)
    w3 = nc.declare_dram_parameter("w3", [HIDDEN, FFN], bf16, isOutput=False)
    w2 = nc.declare_dram_parameter("w2", [FFN, HIDDEN], bf16, isOutput=False)
    if USE_RS:
        y = nc.declare_dram_parameter("y", [H_SHARD, TOKENS], f32, isOutput=True)
    else:
        y = nc.declare_dram_parameter("y", [HIDDEN, TOKENS], f32, isOutput=True)

    # DRAM-side rearranged views (partition-first) for strided loads.
    xtf_v = xtf.rearrange("(k p) t -> p k t", p=128)     # [128, KH, TOKENS]
    xtb_v = xtb.rearrange("(k p) t -> p k t", p=128)
    gw_v = gw.rearrange("(k p) e -> p k e", p=128)       # [128, KH, 8]
    w1_v = w1.rearrange("(k p) m -> p k m", p=128)       # [128, KH, FFN]
    w3_v = w3.rearrange("(k p) m -> p k m", p=128)
    w2_v = w2.rearrange("(k p) m -> p k m", p=128)       # [128, KF, HIDDEN]

    with tile.TileContext(nc) as tc:
        with tc.tile_pool(name="persist", bufs=1) as persist, \
             tc.tile_pool(name="dram", bufs=2, space="DRAM") as dram:

            # ---- persistent tiles ----
            wb_sb = persist.tile([128, TOKENS], f32, tag="wb")      # weight bcast
            hT = persist.tile([128, KF * T_BLK], bf16, tag="hT")    # [f-chunks x tokens]
            ident = persist.tile([128, 128], f32, tag="ident")
            make_identity(nc, ident[:])
            ones1 = persist.tile([1, 128], f32, tag="ones1")
            nc.vector.memset(ones1[:], 1.0)
            gwt = persist.tile([128, KH * NUM_EXPERTS], f32, tag="gwt")
            nc.sync.dma_start(gwt.rearrange("p (k e) -> p k e", k=KH), gw_v[:])
            sel_sb = persist.tile([128, NUM_EXPERTS], f32, tag="sel")
            nc.sync.dma_start(sel_sb[:], sel[:])
            wrow = persist.tile([1, TOKENS], f32, tag="wrow")

            # ================= phase 0: router =================
            with tc.tile_pool(name="rt", bufs=2) as rt, \
                 tc.tile_pool(name="rtp", bufs=2, space="PSUM") as rtp:
                for t in range(TOKENS // 128):
                    xf = rt.tile([128, KH * 128], f32, tag="xf")
                    nc.sync.dma_start(
                        xf.rearrange("p (k c) -> p k c", k=KH),
                        xtf_v[:, :, t * 128:(t + 1) * 128])
                    lp = rtp.tile([128, NUM_EXPERTS], f32, tag="lp")
                    for k in range(KH):
                        nc.tensor.matmul(
                            lp[:], xf[:, k * 128:(k + 1) * 128],
                            gwt[:, k * NUM_EXPERTS:(k + 1) * NUM_EXPERTS],
                            start=(k == 0), stop=(k == KH - 1))
                    ls = rt.tile([128, NUM_EXPERTS], f32, tag="ls")
                    nc.vector.tensor_copy(ls[:], lp[:])
                    m1 = rt.tile([128, 1], f32, tag="m1")
                    nc.vector.reduce_max(m1[:], ls[:], axis=mybir.AxisListType.X)
                    ge1 = rt.tile([128, NUM_EXPERTS], f32, tag="ge1")
                    nc.vector.tensor_scalar(ge1[:], ls[:], m1[:], None, ALU.is_ge)
                    ls2 = rt.tile([128, NUM_EXPERTS], f32, tag="ls2")
                    # ls2 = ge1 * -1e30 + ls   (mask out the top-1 entry)
                    nc.vector.scalar_tensor_tensor(
                        ls2[:], ge1[:], -1.0e30, ls[:], op0=ALU.mult, op1=ALU.add)
                    m2 = rt.tile([128, 1], f32, tag="m2")
                    nc.vector.reduce_max(m2[:], ls2[:], axis=mybir.AxisListType.X)
                    keep = rt.tile([128, NUM_EXPERTS], f32, tag="keep")
                    nc.vector.tensor_scalar(keep[:], ls[:], m2[:], None, ALU.is_ge)
                    m1n = rt.tile([128, 1], f32, tag="m1n")
                    nc.vector.tensor_scalar_mul(m1n[:], m1[:], -1.0)
                    e1 = rt.tile([128, NUM_EXPERTS], f32, tag="e1")
                    nc.scalar.activation(e1[:], ls[:], AF.Exp, bias=m1n[:])
                    ek = rt.tile([128, NUM_EXPERTS], f32, tag="ek")
                    nc.vector.tensor_mul(ek[:], e1[:], keep[:])
                    den = rt.tile([128, 1], f32, tag="den")
                    nc.vector.reduce_sum(den[:], ek[:], axis=mybir.AxisListType.X)
                    rec = rt.tile([128, 1], f32, tag="rec")
                    nc.vector.reciprocal(rec[:], den[:])
                    ws = rt.tile([128, NUM_EXPERTS], f32, tag="ws")
                    nc.vector.tensor_mul(ws[:], ek[:], sel_sb[:])
                    wcu = rt.tile([128, 1], f32, tag="wcu")
                    nc.vector.reduce_sum(wcu[:], ws[:], axis=mybir.AxisListType.X)
                    wcol = rt.tile([128, 1], f32, tag="wcol")
                    nc.vector.tensor_mul(wcol[:], wcu[:], rec[:])
                    tp = rtp.tile([128, 128], f32, tag="tp")
                    nc.tensor.transpose(tp[:1, :128], wcol[:], ident[:])
                    nc.vector.tensor_copy(wrow[:1, t * 128:(t + 1) * 128], tp[:1, :128])
                # broadcast wrow [1, TOKENS] -> wb_sb [128, TOKENS]
                for s in range(TOKENS // 512):
                    bc = rtp.tile([128, 512], f32, tag="bc")
                    nc.tensor.matmul(
                        bc[:], ones1[:], wrow[:1, s * 512:(s + 1) * 512],
                        start=True, stop=True)
                    nc.vector.tensor_copy(wb_sb[:, s * 512:(s + 1) * 512], bc[:])

            # ================= main loop =================
            with tc.tile_pool(name="xb", bufs=2) as xbp, \
                 tc.tile_pool(name="w13", bufs=8) as w13p, \
                 tc.tile_pool(name="w2p", bufs=2) as w2p, \
                 tc.tile_pool(name="tmp", bufs=2) as tmp, \
                 tc.tile_pool(name="oev", bufs=3) as oev, \
                 tc.tile_pool(name="mps", bufs=2, space="PSUM") as mps:
                for b in range(N_BLOCKS):
                    tsl = slice(b * T_BLK, (b + 1) * T_BLK)
                    xb = xbp.tile([128, KH * T_BLK], bf16, tag="xb")
                    nc.sync.dma_start(
                        xb.rearrange("p (k t) -> p k t", k=KH),
                        xtb_v[:, :, tsl])
                    # ---- h phase ----
                    for f in range(KF):
                        fsl = slice(f * 128, (f + 1) * 128)
                        w1t = w13p.tile([128, KH * 128], bf16, tag="w13")
                        nc.sync.dma_start(
                            w1t.rearrange("p (k c) -> p k c", k=KH),
                            w1_v[:, :, fsl])
                        w3t = w13p.tile([128, KH * 128], bf16, tag="w13")
                        nc.sync.dma_start(
                            w3t.rearrange("p (k c) -> p k c", k=KH),
                            w3_v[:, :, fsl])
                        h1p = mps.tile([128, T_BLK], f32, tag="h1")
                        h3p = mps.tile([128, T_BLK], f32, tag="h3")
                        for k in range(KH):
                            ksl = slice(k * 128, (k + 1) * 128)
                            xsl = slice(k * T_BLK, (k + 1) * T_BLK)
                            nc.tensor.matmul(h1p[:], w1t[:, ksl], xb[:, xsl],
                                             start=(k == 0), stop=(k == KH - 1))
                        for k in range(KH):
                            ksl = slice(k * 128, (k + 1) * 128)
                            xsl = slice(k * T_BLK, (k + 1) * T_BLK)
                            nc.tensor.matmul(h3p[:], w3t[:, ksl], xb[:, xsl],
                                             start=(k == 0), stop=(k == KH - 1))
                        h1s = tmp.tile([128, T_BLK], f32, tag="h1s")
                        nc.scalar.activation(h1s[:], h1p[:], AF.Silu)
                        nc.vector.tensor_mul(
                            hT[:, f * T_BLK:(f + 1) * T_BLK], h1s[:], h3p[:])
                    # ---- w2 phase ----
                    outblk = dram.tile([HIDDEN, T_BLK], f32, tag="outblk")
                    for m in range(KH):
                        msl = slice(m * 128, (m + 1) * 128)
                        w2t = w2p.tile([128, KF * 128], bf16, tag="w2")
                        nc.gpsimd.dma_start(
                            w2t.rearrange("p (k c) -> p k c", k=KF),
                            w2_v[:, :, msl])
                        op = mps.tile([128, T_BLK], f32, tag="o")
                        for kf in range(KF):
                            nc.tensor.matmul(
                                op[:], w2t[:, kf * 128:(kf + 1) * 128],
                                hT[:, kf * T_BLK:(kf + 1) * T_BLK],
                                start=(kf == 0), stop=(kf == KF - 1))
                        os = oev.tile([128, T_BLK], f32, tag="os")
                        nc.vector.tensor_mul(os[:], op[:], wb_sb[:, tsl])
                        nc.gpsimd.dma_start(outblk[msl, :], os[:])
                    if USE_RS:
                        rsout = dram.tile([H_SHARD, T_BLK], f32, tag="rsout")
                        nc.gpsimd.collective_compute(
                            "ReduceScatter", mybir.AluOpType.add,
                            replica_groups=[list(range(N_CORES))],
                            ins=[outblk.opt()], outs=[rsout.opt()])
                        nc.gpsimd.dma_start(y[:, tsl], rsout[:])
                    else:
                        nc.gpsimd.dma_start(y[:, tsl], outblk[:])
    return nc


_CACHED = {}


def _get_nc():
    if "nc" not in _CACHED:
        _apply_bir_patch()
        _CACHED["nc"] = build()
    return _CACHED["nc"]


def _prep_in_maps(inputs):
    bf16 = ml_dtypes.bfloat16
    x = np.ascontiguousarray(np.asarray(inputs["hidden_states"], dtype=np.float32))
    gate = np.ascontiguousarray(np.asarray(inputs["gate_w"], dtype=np.float32))
    w1s = np.asarray(inputs["w1s"])
    w3s = np.asarray(inputs["w3s"])
    w2s = np.asarray(inputs["w2s"])

    xtf = np.ascontiguousarray(x.T)                       # [H, T] f32
    xtb = np.ascontiguousarray(xtf.astype(bf16))          # [H, T] bf16

    in_maps = []
    for c in range(N_CORES):
        sel = np.zeros((128, NUM_EXPERTS), dtype=np.float32)
        sel[:, c] = 1.0
        in_maps.append({
            "xtf": xtf,
            "xtb": xtb,
            "gw": gate,
            "sel": sel,
            "w1": np.ascontiguousarray(w1s[c].astype(bf16)),
            "w3": np.ascontiguousarray(w3s[c].astype(bf16)),
            "w2": np.ascontiguousarray(w2s[c].astype(bf16)),
        })
    return in_maps


def run(inputs, trace=False, trace_cores=None):
    """Build + run; returns (full_output [T, H] f32, BassKernelResults)."""
    from concourse.bass_utils import run_bass_kernel_spmd

    nc = _get_nc()
    in_maps = _prep_in_maps(inputs)
    res = run_bass_kernel_spmd(
        nc, in_maps, list(range(N_CORES)), trace=trace,
        **({"trace_cores": trace_cores} if trace_cores else {}))
    if USE_RS:
        outT = np.concatenate([res.results[c]["y"] for c in range(N_CORES)], axis=0)
    else:
        outT = np.sum([res.results[c]["y"] for c in range(N_CORES)], axis=0)
    out = np.ascontiguousarray(outT.T.astype(np.float32))
    return out, res


def kernel(**inputs) -> np.ndarray:
    out, _ = run(inputs, trace=False)
    return out
